# revision 1
# baseline (speedup 1.0000x reference)
"""Trainium2 Bass kernel for nn_AttentionMLP (pairwise-MLP attention + softmax).

Math (per batch b):
  hA = inputA[b] @ W1[:128]          # (K, H)
  hB = inputB[b] @ W1[128:]          # (L, H)
  scores[k, l] = sum_h relu(hA[k, h] + hB[l, h] + b1[h]) * w2[h]
  out[b, k, :] = softmax(scores[k, :])

Shapes: B=2, K=128, L=4096, D=H=128.

Distribution: pure data parallel over the (b, k) grid — core c handles
b = c // 4 and a 32-wide k block (no collectives; the softmax over L is
core-local).

Per-core device algorithm (SBUF partition axis = H):
  1. hBT = W1b.T @ inputB[b].T -> PSUM, copied to SBUF as bf16 [128, 4096]
     (inputs pre-transposed on host so the contraction dim lands on
     partitions).
  2. bias[:, k] = W1a.T @ inputA[b].T + b1  (fp32 [128, 32])
  3. Per k: R_k = relu(hBT + bias[:, k]) as one [128, 4096] bf16 pass —
     VectorE tensor_scalar (add+max, 4x mode) or ScalarE activation
     (per-partition bias), statically load-balanced ~26/6.
  4. scores = w2.T @ R_k via M=32 matmuls (N=512 chunks) whose weight
     matrix is a 32-wide slice of a zeros|w2|zeros band — w2 lands in
     column 4*(k%8)+(chunk//2), so chunk c of k accumulates into PSUM
     partition 4*k + c//2, columns 512*(c%2):...  All 256 matmuls
     accumulate into ONE [128, 1024] PSUM tile (2 banks) holding the
     scores in softmax layout: partition 4k+q = l-range [1024q:1024(q+1)).
     Four matmuls run concurrently via PE col-tiling (tile_position=
     (0,32j), k's 8 apart), so PE streams ~4 cols/cycle.
  5. Softmax without max-subtraction (scores are O(1)): ScalarE exp reads
     the PSUM tile directly (this is also the PSUM evacuation) with
     accum_out producing row sums; per-k sums = quarter sums combined and
     broadcast back via tiny 0/1 matmuls; final scale on VectorE; one
     output DMA.
"""

import os
import sys

for _p in ("/opt/trn_rl_repo", "/root/.axon_site/_ro/trn_rl_repo"):
    if os.path.isdir(_p) and _p not in sys.path:
        sys.path.insert(0, _p)

import numpy as np
import ml_dtypes

BF = ml_dtypes.bfloat16
B, K, L, D, H = 2, 128, 4096, 128, 128
NCORES = 8
KPC = 32   # k's per core
NG = 8     # concurrency groups; group g = k's {g, 8+g, 16+g, 24+g}

import json as _json
# which (g, j) relu passes run on ScalarE (the rest on VectorE)
_AP_ENV = os.environ.get("KERNEL_ACT_PASSES")
if _AP_ENV:
    ACT_PASSES = {tuple(p) for p in _json.loads(_AP_ENV)}
else:
    ACT_PASSES = {(0, 3), (1, 1), (2, 2), (3, 3), (4, 0), (5, 1), (6, 2),
                  (7, 3)}
# which hbt chunk copies run on ScalarE (rest on VectorE)
_HC_ENV = os.environ.get("KERNEL_ACT_COPIES")
if _HC_ENV:
    ACT_COPIES = set(_json.loads(_HC_ENV))
else:
    ACT_COPIES = {1, 3, 5, 7}

_BUILT = None


def _build(reps=1, loop=False):
    import concourse.mybir as mybir
    import concourse.tile as tile
    from concourse import bacc

    dt = mybir.dt
    f32, bf = dt.float32, dt.bfloat16
    AF = mybir.ActivationFunctionType
    ALU = mybir.AluOpType

    nc = bacc.Bacc("TRN2", target_bir_lowering=False, debug=False,
                   enable_asserts=True)

    xbt = nc.dram_tensor("xbt", [128, L], bf, kind="ExternalInput").ap()
    xat = nc.dram_tensor("xat", [128, KPC], bf, kind="ExternalInput").ap()
    w1a = nc.dram_tensor("w1a", [128, H], bf, kind="ExternalInput").ap()
    w1b = nc.dram_tensor("w1b", [128, H], bf, kind="ExternalInput").ap()
    b1c = nc.dram_tensor("b1c", [128, 1], f32, kind="ExternalInput").ap()
    wband = nc.dram_tensor("wband", [128, 64], bf, kind="ExternalInput").ap()
    wcomb = nc.dram_tensor("wcomb", [128, KPC], f32, kind="ExternalInput").ap()
    wbcast = nc.dram_tensor("wbcast", [KPC, 128], f32, kind="ExternalInput").ap()
    out = nc.dram_tensor("out", [128, 1024], f32, kind="ExternalOutput").ap()

    with tile.TileContext(nc) as tc:
        with (
            tc.tile_pool(name="consts", bufs=1) as consts,
            tc.tile_pool(name="work", bufs=1) as work,
            tc.tile_pool(name="rpool", bufs=12) as rpool,
            tc.tile_pool(name="psum", bufs=2, space="PSUM") as psum,
            tc.tile_pool(name="epsum", bufs=1, space="PSUM") as epsum,
        ):
            w1a_sb = consts.tile([128, H], bf, tag="w1a")
            nc.sync.dma_start(w1a_sb[:], w1a)
            w1b_sb = consts.tile([128, H], bf, tag="w1b")
            nc.sync.dma_start(w1b_sb[:], w1b)
            xat_sb = consts.tile([128, KPC], bf, tag="xat")
            nc.sync.dma_start(xat_sb[:], xat)
            b1_sb = consts.tile([128, 1], f32, tag="b1")
            nc.sync.dma_start(b1_sb[:], b1c)
            wband_sb = consts.tile([128, 64], bf, tag="wband")
            nc.sync.dma_start(wband_sb[:], wband)
            wcomb_sb = consts.tile([128, KPC], f32, tag="wcomb")
            nc.sync.dma_start(wcomb_sb[:], wcomb)
            wbcast_sb = consts.tile([KPC, 128], f32, tag="wbcast")
            nc.sync.dma_start(wbcast_sb[:], wbcast)
            # dummy ACT op issued first so the ~2.7us activation-table load
            # overlaps the input DMAs instead of stalling the first real
            # ScalarE op
            warm_sb = consts.tile([128, 1], f32, tag="warm")
            nc.vector.memset(warm_sb[:], 0.0)
            # Exp anchors the exp_and_others table set, which also holds
            # Relu and Copy — one table load serves the whole kernel
            nc.scalar.activation(warm_sb[:], warm_sb[:], AF.Exp)

            if loop and reps > 1:
                with tc.For_i(0, reps, 1):
                    _body(nc, tc, work, rpool, psum, epsum, xbt, out,
                          w1a_sb, w1b_sb, xat_sb, b1_sb, wband_sb, wcomb_sb,
                          wbcast_sb, f32, bf, AF, ALU)
            else:
                for _rep in range(reps):
                    _body(nc, tc, work, rpool, psum, epsum, xbt, out,
                          w1a_sb, w1b_sb, xat_sb, b1_sb, wband_sb, wcomb_sb,
                          wbcast_sb, f32, bf, AF, ALU)

    nc.compile()
    return nc


def _body(nc, tc, work, rpool, psum, epsum, xbt, out,
          w1a_sb, w1b_sb, xat_sb, b1_sb, wband_sb, wcomb_sb, wbcast_sb,
          f32, bf, AF, ALU):
            xbt_sb = work.tile([128, L], bf, tag="xbt")
            for c in range(8):
                nc.sync.dma_start(xbt_sb[:, 512 * c:512 * (c + 1)],
                                  xbt[:, 512 * c:512 * (c + 1)])

            # bias matrix: hAT + b1  (fp32 [128, KPC])
            ps_h = psum.tile([128, 512], f32, tag="ps")
            nc.tensor.matmul(ps_h[:, 0:KPC], lhsT=w1a_sb[:], rhs=xat_sb[:],
                             start=True, stop=True)
            bias_sb = work.tile([128, KPC], f32, tag="bias")
            nc.vector.tensor_scalar(out=bias_sb[:], in0=ps_h[:, 0:KPC],
                                    scalar1=b1_sb[:, 0:1], scalar2=None,
                                    op0=ALU.add)

            # hBT in bf16 SBUF; copies split ACT/DVE
            hbt_sb = work.tile([128, L], bf, tag="hbt")
            for c in range(8):
                ps_c = psum.tile([128, 512], f32, tag="ps")
                sl = slice(512 * c, 512 * c + 512)
                nc.tensor.matmul(ps_c[:], lhsT=w1b_sb[:], rhs=xbt_sb[:, sl],
                                 start=True, stop=True)
                if c in ACT_COPIES:
                    nc.scalar.copy(hbt_sb[:, sl], ps_c[:])
                else:
                    nc.vector.tensor_copy(hbt_sb[:, sl], ps_c[:])

            # scores accumulate into one [128, 1024] PSUM tile:
            # partition 4k+q holds l-range [1024q : 1024(q+1))
            e_ps = epsum.tile([128, 1024], f32, tag="eps")

            for g in range(NG):
                rts = []
                for j in range(4):
                    k = 8 * j + g
                    rt = rpool.tile([128, L], bf, tag="r")
                    # group 0 passes split in halves: the first half only
                    # needs hbt chunks 0-3, so R production (and the PE)
                    # starts ~2us earlier in the single-shot execution
                    halves = ((0, 2048), (2048, 4096)) if g == 0 \
                        else ((0, 4096),)
                    for lo, hi in halves:
                        if (g, j) in ACT_PASSES:
                            nc.scalar.activation(rt[:, lo:hi],
                                                 hbt_sb[:, lo:hi], AF.Relu,
                                                 bias=bias_sb[:, k:k + 1],
                                                 scale=1.0)
                        else:
                            nc.vector.tensor_scalar(
                                out=rt[:, lo:hi], in0=hbt_sb[:, lo:hi],
                                scalar1=bias_sb[:, k:k + 1], scalar2=0.0,
                                op0=ALU.add, op1=ALU.max)
                    rts.append(rt)
                # q-major: one weight slice serves 8 matmuls; in the last
                # group run all win=0 matmuls first so the exp on the first
                # PSUM bank can overlap the win=1 matmuls
                if g < NG - 1:
                    order = [(2 * q + win, j) for q in range(4)
                             for j in range(4) for win in range(2)]
                else:
                    order = ([(2 * q, j) for q in range(4) for j in range(4)]
                             + [(2 * q + 1, j) for q in range(4)
                                for j in range(4)])
                for c, j in order:
                    win = c % 2
                    v = 4 * g + c // 2  # local column for w2
                    nc.tensor.matmul(
                        e_ps[32 * j:32 * j + 32,
                             512 * win:512 * win + 512],
                        lhsT=wband_sb[:, 31 - v:63 - v],
                        rhs=rts[j][:, 512 * c:512 * c + 512],
                        start=(g == 0 and c // 2 == 0),
                        stop=(g == NG - 1 and c // 2 == 3),
                        tile_position=(0, 32 * j),
                        skip_group_check=True)

            # softmax tail; exp directly off PSUM = evacuation.
            # Two halves (by PSUM bank) so exp of bank 0 overlaps the
            # win=1 matmuls of the last group.
            e2_sb = work.tile([128, 1024], f32, tag="exp")
            s0_sb = work.tile([128, 1], f32, tag="sums0")
            s1_sb = work.tile([128, 1], f32, tag="sums1")
            nc.scalar.activation(e2_sb[:, 0:512], e_ps[:, 0:512], AF.Exp,
                                 accum_out=s0_sb[:, 0:1])
            nc.scalar.activation(e2_sb[:, 512:1024], e_ps[:, 512:1024],
                                 AF.Exp, accum_out=s1_sb[:, 0:1])
            # combine the two half-sums via PSUM accumulation; the s0 matmul
            # runs while the win=1 exp is still in flight
            ps_t = psum.tile([128, 512], f32, tag="ps")
            nc.tensor.matmul(ps_t[0:KPC, 0:1], lhsT=wcomb_sb[:],
                             rhs=s0_sb[:, 0:1], start=True, stop=False)
            nc.tensor.matmul(ps_t[0:KPC, 0:1], lhsT=wcomb_sb[:],
                             rhs=s1_sb[:, 0:1], start=False, stop=True)
            tr_sb = work.tile([KPC, 1], f32, tag="recip")
            nc.vector.reciprocal(tr_sb[:], ps_t[0:KPC, 0:1])
            ps_u = psum.tile([128, 512], f32, tag="ps")
            nc.tensor.matmul(ps_u[:, 0:1], lhsT=wbcast_sb[:], rhs=tr_sb[:],
                             start=True, stop=True)
            f_sb = work.tile([128, 1024], f32, tag="final")
            nc.vector.tensor_scalar_mul(out=f_sb[:, 0:512],
                                        in0=e2_sb[:, 0:512],
                                        scalar1=ps_u[:, 0:1])
            nc.sync.dma_start(out[:, 0:512], f_sb[:, 0:512])
            nc.vector.tensor_scalar_mul(out=f_sb[:, 512:1024],
                                        in0=e2_sb[:, 512:1024],
                                        scalar1=ps_u[:, 0:1])
            nc.sync.dma_start(out[:, 512:1024], f_sb[:, 512:1024])


def _get_built():
    global _BUILT
    if _BUILT is None:
        _BUILT = _build()
    return _BUILT


def make_in_maps(inputA, inputB, W1, b1, w2):
    wband = np.zeros((128, 64), np.float32)
    wband[:, 31] = w2
    wcomb = (np.arange(128)[:, None] // 4 == np.arange(KPC)[None, :]) \
        .astype(np.float32)
    wbcast = (np.arange(128)[None, :] // 4 == np.arange(KPC)[:, None]) \
        .astype(np.float32)
    w1a = np.ascontiguousarray(W1[:D]).astype(BF)
    w1b = np.ascontiguousarray(W1[D:]).astype(BF)
    b1c = np.ascontiguousarray(b1.reshape(128, 1)).astype(np.float32)
    wband = wband.astype(BF)
    in_maps = []
    for core in range(NCORES):
        b, kq = core // 4, core % 4
        k0 = KPC * kq
        in_maps.append({
            "xbt": np.ascontiguousarray(inputB[b].T).astype(BF),
            "xat": np.ascontiguousarray(inputA[b, k0:k0 + KPC].T).astype(BF),
            "w1a": w1a, "w1b": w1b, "b1c": b1c, "wband": wband,
            "wcomb": wcomb, "wbcast": wbcast,
        })
    return in_maps


def assemble(results):
    """results: list of 8 dicts with 'out' [128, 1024] f32."""
    full = np.empty((B, K, L), np.float32)
    for core in range(NCORES):
        b, kq = core // 4, core % 4
        full[b, KPC * kq:KPC * (kq + 1)] = \
            np.asarray(results[core]["out"]).reshape(KPC, L)
    return full


def kernel(**inputs):
    from concourse.bass_utils import run_bass_kernel_spmd

    inputA = np.asarray(inputs["inputA"], np.float32)
    inputB = np.asarray(inputs["inputB"], np.float32)
    W1 = np.asarray(inputs["W1"], np.float32)
    b1 = np.asarray(inputs["b1"], np.float32)
    w2 = np.asarray(inputs["w2"], np.float32)

    nc = _get_built()
    in_maps = make_in_maps(inputA, inputB, W1, b1, w2)
    res = run_bass_kernel_spmd(nc, in_maps, core_ids=list(range(NCORES)))
    return assemble(res.results)



# revision 26
# speedup vs baseline: 1.7705x; 1.7705x over previous
"""Trainium2 Bass kernel for nn_AttentionMLP (pairwise-MLP attention + softmax).

Math (per batch b):
  hA = inputA[b] @ W1[:128]          # (K, H)
  hB = inputB[b] @ W1[128:]          # (L, H)
  scores[k, l] = sum_h relu(hA[k, h] + hB[l, h] + b1[h]) * w2[h]
  out[b, k, :] = softmax(scores[k, :])

Shapes: B=2, K=128, L=4096, D=H=128.

Distribution: pure data parallel over the (b, k) grid — core c handles
b = c // 4 and a 32-wide k block (no collectives; the softmax over L is
core-local).

Per-core device algorithm (SBUF partition axis = H), v2:
  1. hBT = W1b.T @ inputB[b].T -> PSUM, copied to SBUF as bf16 [128, 4096]
     (inputs pre-transposed on host so the contraction dim lands on
     partitions). Input DMA fanned out over the SP and ACT HWDGE queues.
  2. bias[:, k] = W1a.T @ inputA[b].T + b1  (fp32 [128, 32])
  3. Per k: R_k = relu(hBT + bias[:, k]) as one [128, 4096] bf16 pass —
     VectorE tensor_scalar (add+max, 4x mode) for 24 k's, ScalarE
     activation (per-partition bias) for 8, statically load-balanced.
     (GpSimd/Pool tensor_scalar was tried and measured ~50x slower than
     the cost model predicts — ~57us per pass — so Pool gets none.)
  4. scores = w2.T @ R_k via M=32 matmuls (N=512 chunks) whose weight
     matrix is a 32-wide slice of a zeros|w2|zeros band — w2 lands in
     column 4*(k%8)+(chunk//2), so chunk c of k accumulates into PSUM
     partition 4*k + c//2, columns 512*(c%2):...  All 256 matmuls
     accumulate into ONE [128, 1024] PSUM tile (2 banks) holding the
     scores in softmax layout: partition 4k+q = l-range [1024q:1024(q+1)).
     Four matmuls run concurrently via PE col-tiling (tile_position=
     (0,32j), k's 8 apart).
  5. Softmax without max-subtraction (scores are O(1)): ScalarE exp reads
     the PSUM tile directly (this is also the PSUM evacuation) with
     accum_out producing row sums; per-k sums = quarter sums combined and
     broadcast back via tiny 0/1 matmuls; final scale on VectorE; output
     DMA split over the SP and ACT queues.

All per-iteration tiles live in bufs=2 pools and the hardware loop runs
two body copies per For_i iteration, so consecutive iterations
double-buffer: iteration i+1's input DMA + hBT production overlap
iteration i's relu/score/softmax work.
"""

import os
import sys

for _p in ("/opt/trn_rl_repo", "/root/.axon_site/_ro/trn_rl_repo"):
    if os.path.isdir(_p) and _p not in sys.path:
        sys.path.insert(0, _p)

import numpy as np
import ml_dtypes

BF = ml_dtypes.bfloat16
B, K, L, D, H = 2, 128, 4096, 128, 128
NCORES = 8
KPC = 32   # k's per core
NG = 8     # concurrency groups; group g = k's {g, 8+g, 16+g, 24+g}

import json as _json


def _env_pairs(name, default):
    v = os.environ.get(name)
    return {tuple(p) for p in _json.loads(v)} if v else set(default)


def _env_set(name, default):
    v = os.environ.get(name)
    return set(_json.loads(v)) if v else set(default)


# relu passes on ScalarE / GpSimd (rest on VectorE).  GpSimd's
# tensor_scalar measured ~50x slower than the cost model predicts
# (~57us per [128,4096] pass), so POOL_PASSES defaults to empty.
ACT_PASSES = _env_pairs("KERNEL_ACT_PASSES",
                        [(0, 3), (1, 1), (2, 2), (3, 3), (4, 0), (5, 1),
                         (6, 2), (7, 3)])
POOL_PASSES = _env_pairs("KERNEL_POOL_PASSES", [])
# hbt chunk copies on ScalarE (rest on VectorE)
ACT_COPIES = _env_set("KERNEL_ACT_COPIES", [1, 5])
# ablation variant: base | peonly | reluonly | skeleton
VARIANT = os.environ.get("KERNEL_VARIANT", "base")
# fan input/output DMAs across SP+ACT queues (else all SP)
DMA_SPLIT = os.environ.get("KERNEL_DMA_SPLIT", "1") == "1"
# loop body order: 1 = H(0) H(1) T(0) T(1), 0 = H(0) T(0) H(1) T(1)
PIPE_TAILS = os.environ.get("KERNEL_PIPE_TAILS", "0") == "1"
# second DMA queue: act (HWDGE) or pool (SWDGE)
DMA_QUEUE2 = os.environ.get("KERNEL_DMA_QUEUE2", "act")

_BUILT = None


def _build(reps=1, loop=False, act_passes=None, pool_passes=None,
           act_copies=None):
    global ACT_PASSES, POOL_PASSES, ACT_COPIES
    if act_passes is not None:
        ACT_PASSES = {tuple(p) for p in act_passes}
    if pool_passes is not None:
        POOL_PASSES = {tuple(p) for p in pool_passes}
    if act_copies is not None:
        ACT_COPIES = set(act_copies)
    import concourse.mybir as mybir
    import concourse.tile as tile
    from concourse import bacc

    dt = mybir.dt
    f32, bf = dt.float32, dt.bfloat16
    AF = mybir.ActivationFunctionType
    ALU = mybir.AluOpType

    nc = bacc.Bacc("TRN2", target_bir_lowering=False, debug=False,
                   enable_asserts=True)

    xbt = nc.dram_tensor("xbt", [128, L], bf, kind="ExternalInput").ap()
    xat = nc.dram_tensor("xat", [128, KPC], bf, kind="ExternalInput").ap()
    w1a = nc.dram_tensor("w1a", [128, H], bf, kind="ExternalInput").ap()
    w1b = nc.dram_tensor("w1b", [128, H], bf, kind="ExternalInput").ap()
    b1c = nc.dram_tensor("b1c", [128, 1], f32, kind="ExternalInput").ap()
    wband = nc.dram_tensor("wband", [128, 64], bf, kind="ExternalInput").ap()
    wcomb = nc.dram_tensor("wcomb", [128, KPC], f32, kind="ExternalInput").ap()
    wbcast = nc.dram_tensor("wbcast", [KPC, 128], f32, kind="ExternalInput").ap()
    out = nc.dram_tensor("out", [128, 1024], f32, kind="ExternalOutput").ap()

    with tile.TileContext(nc) as tc:
        with (
            tc.tile_pool(name="consts", bufs=1) as consts,
            tc.tile_pool(name="work", bufs=2) as work,
            tc.tile_pool(name="rpool", bufs=int(os.environ.get("KERNEL_RBUFS", "12"))) as rpool,
            tc.tile_pool(name="psum", bufs=4, space="PSUM") as psum,
            tc.tile_pool(name="epsum", bufs=2, space="PSUM") as epsum,
        ):
            w1a_sb = consts.tile([128, H], bf, tag="w1a")
            nc.sync.dma_start(w1a_sb[:], w1a)
            w1b_sb = consts.tile([128, H], bf, tag="w1b")
            nc.sync.dma_start(w1b_sb[:], w1b)
            xat_sb = consts.tile([128, KPC], bf, tag="xat")
            nc.sync.dma_start(xat_sb[:], xat)
            b1_sb = consts.tile([128, 1], f32, tag="b1")
            nc.sync.dma_start(b1_sb[:], b1c)
            wband_sb = consts.tile([128, 64], bf, tag="wband")
            nc.sync.dma_start(wband_sb[:], wband)
            wcomb_sb = consts.tile([128, KPC], f32, tag="wcomb")
            nc.sync.dma_start(wcomb_sb[:], wcomb)
            wbcast_sb = consts.tile([KPC, 128], f32, tag="wbcast")
            nc.sync.dma_start(wbcast_sb[:], wbcast)
            # dummy ACT op issued first so the ~2.7us activation-table load
            # overlaps the input DMAs instead of stalling the first real
            # ScalarE op.  Exp anchors the exp_and_others table set, which
            # also holds Relu and Copy — one table load serves the kernel.
            warm_sb = consts.tile([128, 1], f32, tag="warm")
            nc.vector.memset(warm_sb[:], 0.0)
            nc.scalar.activation(warm_sb[:], warm_sb[:], AF.Exp)

            args = (nc, work, rpool, psum, epsum, xbt, out, w1a_sb, w1b_sb,
                    xat_sb, b1_sb, wband_sb, wcomb_sb, wbcast_sb, f32, bf,
                    AF, ALU)
            if loop and reps > 1:
                assert reps % 2 == 0
                # software-pipelined: both heads issue before either tail,
                # so each engine's queue stays busy with body i+1's relu
                # while body i's PE drains toward the exp
                with tc.For_i(0, reps // 2, 1):
                    if PIPE_TAILS:
                        c0 = _head(*args)
                        c1 = _head(*args)
                        _tail(c0)
                        _tail(c1)
                    else:
                        _tail(_head(*args))
                        _tail(_head(*args))
            else:
                for _rep in range(reps):
                    _tail(_head(*args))

    nc.compile()
    return nc


def _head(nc, work, rpool, psum, epsum, xbt, out, w1a_sb, w1b_sb, xat_sb,
          b1_sb, wband_sb, wcomb_sb, wbcast_sb, f32, bf, AF, ALU):
    # --- head: input DMA (4 chunks over 2 HWDGE queues) + hBT + bias ---
    eng2 = nc.gpsimd if DMA_QUEUE2 == "pool" else nc.scalar
    xbt_sb = work.tile([128, L], bf, tag="xbt")
    for c in range(4):
        eng = eng2 if (DMA_SPLIT and c % 2 == 1) else nc.sync
        eng.dma_start(xbt_sb[:, 1024 * c:1024 * (c + 1)],
                      xbt[:, 1024 * c:1024 * (c + 1)])

    ps_h = psum.tile([128, 512], f32, tag="ps")
    nc.tensor.matmul(ps_h[:, 0:KPC], lhsT=w1a_sb[:], rhs=xat_sb[:],
                     start=True, stop=True)
    bias_sb = work.tile([128, KPC], f32, tag="bias")
    nc.vector.tensor_scalar(out=bias_sb[:], in0=ps_h[:, 0:KPC],
                            scalar1=b1_sb[:, 0:1], scalar2=None,
                            op0=ALU.add)

    hbt_sb = work.tile([128, L], bf, tag="hbt")
    for c in range(8):
        ps_c = psum.tile([128, 512], f32, tag="ps")
        sl = slice(512 * c, 512 * c + 512)
        nc.tensor.matmul(ps_c[:], lhsT=w1b_sb[:], rhs=xbt_sb[:, sl],
                         start=True, stop=True)
        if c in ACT_COPIES:
            nc.scalar.copy(hbt_sb[:, sl], ps_c[:])
        else:
            nc.vector.tensor_copy(hbt_sb[:, sl], ps_c[:])

    # --- scores: relu passes (3 engines) + banded matmuls into PSUM ---
    # partition 4k+q holds l-range [1024q : 1024(q+1))
    e_ps = epsum.tile([128, 1024], f32, tag="eps")

    pe_ablate = VARIANT in ("peonly", "skeleton")
    for g in range(NG):
        if VARIANT == "skeleton":
            break
        rts = []
        for j in range(4):
            k = 8 * j + g
            if pe_ablate:
                rts.append(hbt_sb)
                continue
            rt = rpool.tile([128, L], bf, tag="r")
            # group 0 passes split in halves: the first half only needs
            # hbt chunks 0-3, so R production (and the PE) starts earlier
            # in the single-shot execution
            halves = ((0, 2048), (2048, 4096)) if g == 0 else ((0, 4096),)
            for lo, hi in halves:
                if (g, j) in ACT_PASSES:
                    nc.scalar.activation(rt[:, lo:hi], hbt_sb[:, lo:hi],
                                         AF.Relu,
                                         bias=bias_sb[:, k:k + 1],
                                         scale=1.0)
                elif (g, j) in POOL_PASSES:
                    nc.gpsimd.tensor_scalar(
                        out=rt[:, lo:hi], in0=hbt_sb[:, lo:hi],
                        scalar1=bias_sb[:, k:k + 1], scalar2=0.0,
                        op0=ALU.add, op1=ALU.max)
                else:
                    nc.vector.tensor_scalar(
                        out=rt[:, lo:hi], in0=hbt_sb[:, lo:hi],
                        scalar1=bias_sb[:, k:k + 1], scalar2=0.0,
                        op0=ALU.add, op1=ALU.max)
            rts.append(rt)
        if VARIANT == "reluonly":
            continue
        # q-major: one weight slice serves 8 matmuls; in the last group
        # run all win=0 matmuls first so the exp on the first PSUM bank
        # can overlap the win=1 matmuls
        if g < NG - 1:
            order = [(2 * q + win, j) for q in range(4)
                     for j in range(4) for win in range(2)]
        else:
            order = ([(2 * q, j) for q in range(4) for j in range(4)]
                     + [(2 * q + 1, j) for q in range(4) for j in range(4)])
        for c, j in order:
            win = c % 2
            v = 4 * g + c // 2  # local column for w2
            nc.tensor.matmul(
                e_ps[32 * j:32 * j + 32, 512 * win:512 * win + 512],
                lhsT=wband_sb[:, 31 - v:63 - v],
                rhs=rts[j][:, 512 * c:512 * c + 512],
                start=(g == 0 and c // 2 == 0),
                stop=(g == NG - 1 and c // 2 == 3),
                tile_position=(0, 32 * j),
                skip_group_check=True)
    if VARIANT in ("reluonly", "skeleton"):
        nc.vector.memset(e_ps[:, 0:1024], 0.0)

    return (nc, work, psum, out, wcomb_sb, wbcast_sb, f32, AF, e_ps)


def _tail(ctx):
    nc, work, psum, out, wcomb_sb, wbcast_sb, f32, AF, e_ps = ctx

    # --- softmax tail; exp directly off PSUM = evacuation ---
    e2_sb = work.tile([128, 1024], f32, tag="exp")
    s0_sb = work.tile([128, 1], f32, tag="sums0")
    s1_sb = work.tile([128, 1], f32, tag="sums1")
    nc.scalar.activation(e2_sb[:, 0:512], e_ps[:, 0:512], AF.Exp,
                         accum_out=s0_sb[:, 0:1])
    nc.scalar.activation(e2_sb[:, 512:1024], e_ps[:, 512:1024], AF.Exp,
                         accum_out=s1_sb[:, 0:1])
    # combine the two half-sums via PSUM accumulation
    ps_t = psum.tile([128, 512], f32, tag="ps")
    nc.tensor.matmul(ps_t[0:KPC, 0:1], lhsT=wcomb_sb[:], rhs=s0_sb[:, 0:1],
                     start=True, stop=False)
    nc.tensor.matmul(ps_t[0:KPC, 0:1], lhsT=wcomb_sb[:], rhs=s1_sb[:, 0:1],
                     start=False, stop=True)
    tr_sb = work.tile([KPC, 1], f32, tag="recip")
    nc.vector.reciprocal(tr_sb[:], ps_t[0:KPC, 0:1])
    ps_u = psum.tile([128, 512], f32, tag="ps")
    nc.tensor.matmul(ps_u[:, 0:1], lhsT=wbcast_sb[:], rhs=tr_sb[:],
                     start=True, stop=True)
    f_sb = work.tile([128, 1024], f32, tag="final")
    nc.vector.tensor_scalar_mul(out=f_sb[:, 0:512], in0=e2_sb[:, 0:512],
                                scalar1=ps_u[:, 0:1])
    nc.sync.dma_start(out[:, 0:512], f_sb[:, 0:512])
    nc.vector.tensor_scalar_mul(out=f_sb[:, 512:1024],
                                in0=e2_sb[:, 512:1024],
                                scalar1=ps_u[:, 0:1])
    eng2 = nc.gpsimd if DMA_QUEUE2 == "pool" else nc.scalar
    (eng2 if DMA_SPLIT else nc.sync).dma_start(out[:, 512:1024],
                                               f_sb[:, 512:1024])


def _get_built():
    global _BUILT
    if _BUILT is None:
        _BUILT = _build()
    return _BUILT


def make_in_maps(inputA, inputB, W1, b1, w2):
    wband = np.zeros((128, 64), np.float32)
    wband[:, 31] = w2
    wcomb = (np.arange(128)[:, None] // 4 == np.arange(KPC)[None, :]) \
        .astype(np.float32)
    wbcast = (np.arange(128)[None, :] // 4 == np.arange(KPC)[:, None]) \
        .astype(np.float32)
    w1a = np.ascontiguousarray(W1[:D]).astype(BF)
    w1b = np.ascontiguousarray(W1[D:]).astype(BF)
    b1c = np.ascontiguousarray(b1.reshape(128, 1)).astype(np.float32)
    wband = wband.astype(BF)
    in_maps = []
    for core in range(NCORES):
        b, kq = core // 4, core % 4
        k0 = KPC * kq
        in_maps.append({
            "xbt": np.ascontiguousarray(inputB[b].T).astype(BF),
            "xat": np.ascontiguousarray(inputA[b, k0:k0 + KPC].T).astype(BF),
            "w1a": w1a, "w1b": w1b, "b1c": b1c, "wband": wband,
            "wcomb": wcomb, "wbcast": wbcast,
        })
    return in_maps


def assemble(results):
    """results: list of 8 dicts with 'out' [128, 1024] f32."""
    full = np.empty((B, K, L), np.float32)
    for core in range(NCORES):
        b, kq = core // 4, core % 4
        full[b, KPC * kq:KPC * (kq + 1)] = \
            np.asarray(results[core]["out"]).reshape(KPC, L)
    return full


def kernel(**inputs):
    from concourse.bass_utils import run_bass_kernel_spmd

    inputA = np.asarray(inputs["inputA"], np.float32)
    inputB = np.asarray(inputs["inputB"], np.float32)
    W1 = np.asarray(inputs["W1"], np.float32)
    b1 = np.asarray(inputs["b1"], np.float32)
    w2 = np.asarray(inputs["w2"], np.float32)

    nc = _get_built()
    in_maps = make_in_maps(inputA, inputB, W1, b1, w2)
    res = run_bass_kernel_spmd(nc, in_maps, core_ids=list(range(NCORES)))
    return assemble(res.results)
